# revision 8
# baseline (speedup 1.0000x reference)
"""Grouped-Query Attention kernel for 8 Trainium2 NeuronCores.

Problem (hardcoded): x [2,2048,2048] f32, Wq [2048,2048], Wk/Wv [512,2048],
Wo [2048,2048]; 32 q-heads, 8 kv-heads, head_dim 64, causal softmax.

Sharding: hybrid tensor-parallel = (batch 2) x (head-group 4). Core c handles
batch b=c//4, head-group g=c%4: 8 q-heads, 2 kv-heads, row-parallel Wo slice.
Each core emits a partial yT [H, S]; host sums the 4 partials per batch
(the all-reduce of the row-parallel projection) and transposes back.

All activations live transposed on-chip (feature dim on partitions) so every
matmul has its contraction on the partition axis:
  QT = wq_p.T @ xT   (per head-group, head-permuted columns)
  scoresT[k,q] = (KT.T QT) -- lhsT = KT slice, rhs = QT slice, K=64
  PT = exp(0.125*scoresT + causal mask)
  O' = V_aug.T @ PT  -- V augmented with a ones column => softmax denominator
  yT_partial = woTs_p @ OT_norm

Matmuls run in float32r (TF32-class, ~1.7e-4 rel err, 4x fp32 throughput).
"""

import numpy as np
import concourse.mybir as mybir
import concourse.tile as tile
from concourse import bacc, bass_utils

# ---- problem constants ----
B, S, H = 2, 2048, 2048
NKV, NQ, HD = 8, 32, 64
G = NQ // NKV              # 4 q-heads per kv-head
N_CORES = 8
NGROUPS = 4                # head-groups (cores per batch)
DQ = 512                   # q-proj rows per core (8 heads x 64)
DKV = 128                  # kv rows per core (2 kv heads x 64)
SC = 512                   # free-dim chunk (psum bank width in fp32)
NSC = S // SC              # 4
HT = H // 128              # 16 contraction tiles
ST = S // 128              # 16 key tiles
NEG = -1.0e5               # causal mask fill (exp(0.125*-1e5) == 0)

F32 = mybir.dt.float32
F32R = mybir.dt.float32r


def build_gqa(loops: int = 1):
    """Build + bacc-compile the per-core kernel (SPMD: same program, cores
    differ only in input data)."""
    nc = bacc.Bacc("TRN2", target_bir_lowering=False, debug=False,
                   num_devices=N_CORES)

    # DRAM I/O (per-core shapes). float32r tensors carry fp32 bits; the PE
    # rounds them on read.
    xT_d = nc.dram_tensor("xT", [H, S], F32R, kind="ExternalInput")
    wq_d = nc.dram_tensor("wq", [H, DQ], F32R, kind="ExternalInput")
    wk_d = nc.dram_tensor("wk", [H, DKV], F32R, kind="ExternalInput")
    wv_d = nc.dram_tensor("wv", [H, DKV], F32R, kind="ExternalInput")
    wo_d = nc.dram_tensor("wo", [DQ, H], F32R, kind="ExternalInput")
    mask_d = nc.dram_tensor("mask", [128, 384 + SC], F32, kind="ExternalInput")
    ident_d = nc.dram_tensor("ident", [128, 64], F32R, kind="ExternalInput")
    vc0_d = nc.dram_tensor("vc0", [128, ST * 65], F32R, kind="ExternalInput")
    vc1_d = nc.dram_tensor("vc1", [128, ST * 128], F32R, kind="ExternalInput")
    ones_d = nc.dram_tensor("ones", [1, 128], F32R, kind="ExternalInput")
    yT_d = nc.dram_tensor("yT", [H, S], F32, kind="ExternalOutput")

    with tile.TileContext(nc) as tc:
        with tc.tile_pool(name="persist", bufs=1) as pp:
            # persistent SBUF (per-partition bytes in comments)
            qt = [pp.tile([128, S], F32R, tag=f"qt{i}", name=f"qt{i}") for i in range(4)]   # 32K
            kt = pp.tile([128, S], F32R, tag="kt")                            # 8K
            ot = [pp.tile([128, S], F32R, tag=f"ot{i}", name=f"ot{i}") for i in range(4)]   # 32K
            # V_aug: kh=0 layout [v(64) | ones] width 65 (O rows 0-63, den 64)
            #        kh=1 layout [ones | zeros(63) | v(64)] width 128
            #        (den row 0, O rows 64-127)
            vaug0 = pp.tile([128, ST * 65], F32R, tag="vaug0")               # 4.1K
            vaug1 = pp.tile([128, ST * 128], F32R, tag="vaug1")              # 8K
            mask_t = pp.tile([128, 384 + SC], F32, tag="mask")               # 3.5K
            ident_t = pp.tile([128, 64], F32R, tag="ident")                  # 0.25K
            ones_t = pp.tile([1, 128], F32R, tag="ones")                     # tiny

            nc.sync.dma_start(mask_t[:], mask_d[:])
            nc.sync.dma_start(ident_t[:], ident_d[:])
            nc.sync.dma_start(ones_t[:], ones_d[:])
            # constant columns of V_aug (ones/zeros pattern from host)
            nc.sync.dma_start(vaug0[:], vc0_d[:])
            nc.sync.dma_start(vaug1[:], vc1_d[:])

            for _rep in range(loops):
                # ============ Phase 1: QKV projections (+ V transpose) ======
                with (
                    tc.tile_pool(name="p1sb", bufs=1) as p1,
                    tc.tile_pool(name="p1ps", bufs=1, space="PSUM") as ps1,
                ):
                    wq_t = p1.tile([128, HT * DQ], F32R, tag="wq")           # 32K
                    wk_t = p1.tile([128, HT * DKV], F32R, tag="wk")          # 8K
                    wv_t = p1.tile([128, HT * DKV], F32R, tag="wv")          # 8K
                    nc.sync.dma_start(
                        wq_t[:].rearrange("p (t m) -> p t m", t=HT),
                        wq_d[:].rearrange("(t p) m -> p t m", p=128))
                    nc.sync.dma_start(
                        wk_t[:].rearrange("p (t m) -> p t m", t=HT),
                        wk_d[:].rearrange("(t p) m -> p t m", p=128))
                    nc.sync.dma_start(
                        wv_t[:].rearrange("p (t m) -> p t m", t=HT),
                        wv_d[:].rearrange("(t p) m -> p t m", p=128))

                    for sq in range(NSC):
                        xq = [p1.tile([128, SC], F32R, tag="xq", name=f"xq{ht}", bufs=20) for ht in range(HT)]
                        for ht in range(HT):
                            nc.sync.dma_start(
                                xq[ht][:],
                                xT_d[ht * 128:(ht + 1) * 128, sq * SC:(sq + 1) * SC])
                        # 6 output m-tiles: 4x QT, 1x KT, 1x VT
                        vt_tmp = p1.tile([128, SC], F32R, tag="vt", bufs=2)
                        jobs = [(wq_t, DQ, mi, qt[mi]) for mi in range(4)]
                        jobs.append((wk_t, DKV, 0, kt))
                        jobs.append((wv_t, DKV, 0, None))  # None -> vt_tmp
                        for w_t, mw, mi, dest in jobs:
                            psum = ps1.tile([128, SC], F32, tag="qkv", bufs=2)
                            for ht in range(HT):
                                nc.tensor.matmul(
                                    psum[:],
                                    w_t[:, ht * mw + mi * 128: ht * mw + mi * 128 + 128],
                                    xq[ht][:],
                                    start=(ht == 0), stop=(ht == HT - 1))
                            if dest is None:
                                nc.vector.tensor_copy(vt_tmp[:], psum[:])
                            else:
                                nc.vector.tensor_copy(
                                    dest[:, sq * SC:(sq + 1) * SC], psum[:])
                        # V transpose for this quarter's 4 key-tiles
                        for stq in range(4):
                            st = sq * 4 + stq
                            for kh in range(2):
                                tp = ps1.tile([128, 64], F32R, tag="tr", bufs=2)
                                nc.tensor.transpose(
                                    tp[:],
                                    vt_tmp[kh * 64:(kh + 1) * 64,
                                           stq * 128:(stq + 1) * 128],
                                    ident_t[kh * 64:(kh + 1) * 64, :])
                                if kh == 0:
                                    nc.vector.tensor_copy(
                                        vaug0[:, st * 65: st * 65 + 64], tp[:])
                                else:
                                    nc.vector.tensor_copy(
                                        vaug1[:, st * 128 + 64: st * 128 + 128], tp[:])

                # ============ Phase 2+3 pools ===============================
                with (
                    tc.tile_pool(name="p2sb", bufs=1) as p2,
                    tc.tile_pool(name="p2ps", bufs=1, space="PSUM") as ps2,
                ):
                    wo_t = p2.tile([128, 4 * H], F32R, tag="wo")             # 32K
                    nc.sync.dma_start(
                        wo_t[:].rearrange("p (t m) -> p t m", t=4),
                        wo_d[:].rearrange("(t p) m -> p t m", p=128))

                    # ---- Phase 2: causal attention ----
                    for qc in range(NSC):
                        for kh in range(2):
                            for gg in range(4):
                                # head (kh, gg): QT rows gg*128 + kh*64 (host
                                # permuted so base matches kt's kh*64)
                                q_ap = qt[gg][kh * 64:(kh + 1) * 64,
                                              qc * SC:(qc + 1) * SC]
                                o_ps = ps2.tile([128, SC], F32, tag="o", bufs=2)
                                n_st = 4 * (qc + 1)
                                for st in range(n_st):
                                    sc_ps = ps2.tile([128, SC], F32, tag="sc", bufs=2)
                                    nc.tensor.matmul(
                                        sc_ps[:],
                                        kt[kh * 64:(kh + 1) * 64,
                                           st * 128:(st + 1) * 128],
                                        q_ap,
                                        start=True, stop=True)
                                    dd = st - 4 * qc
                                    if dd >= 0:  # diagonal band tile
                                        nc.vector.tensor_add(
                                            sc_ps[:], sc_ps[:],
                                            mask_t[:, 384 - dd * 128:
                                                   896 - dd * 128])
                                    pt = p2.tile([128, SC], F32R, tag="pt", bufs=4)
                                    nc.scalar.activation(
                                        pt[:], sc_ps[:],
                                        mybir.ActivationFunctionType.Exp,
                                        scale=0.125)
                                    if kh == 0:
                                        lhs = vaug0[:, st * 65: st * 65 + 65]
                                        out_ap = o_ps[0:65, :]
                                    else:
                                        lhs = vaug1[:, st * 128: st * 128 + 128]
                                        out_ap = o_ps[:]
                                    nc.tensor.matmul(
                                        out_ap, lhs, pt[:],
                                        start=(st == 0), stop=(st == n_st - 1))
                                # normalize rows by the ones-column sums
                                den_row = o_ps[64:65, :] if kh == 0 else o_ps[0:1, :]
                                o_rows = o_ps[0:64, :] if kh == 0 else o_ps[64:128, :]
                                rec32 = p2.tile([1, SC], F32, tag="rec32", bufs=2)
                                nc.vector.reciprocal(rec32[:], den_row)
                                rec_r = p2.tile([1, SC], F32R, tag="recr", bufs=2)
                                nc.vector.tensor_copy(rec_r[:], rec32[:])
                                rb_ps = ps2.tile([128, SC], F32, tag="rb", bufs=2)
                                nc.tensor.matmul(rb_ps[:], ones_t[:], rec_r[:],
                                                 start=True, stop=True)
                                rb_sb = p2.tile([128, SC], F32, tag="rbsb", bufs=2)
                                nc.vector.tensor_copy(
                                    rb_sb[kh * 64:(kh + 1) * 64, :],
                                    rb_ps[kh * 64:(kh + 1) * 64, :])
                                nc.vector.tensor_mul(
                                    ot[gg][kh * 64:(kh + 1) * 64,
                                           qc * SC:(qc + 1) * SC],
                                    o_rows, rb_sb[kh * 64:(kh + 1) * 64, :])

                    # ---- Phase 3: row-parallel output projection ----
                    for mi in range(HT):
                        for sc_i in range(NSC):
                            y_ps = ps2.tile([128, SC], F32, tag="y", bufs=2)
                            for kt_i in range(4):
                                nc.tensor.matmul(
                                    y_ps[:],
                                    wo_t[:, kt_i * H + mi * 128:
                                         kt_i * H + mi * 128 + 128],
                                    ot[kt_i][:, sc_i * SC:(sc_i + 1) * SC],
                                    start=(kt_i == 0), stop=(kt_i == 3))
                            y_sb = p2.tile([128, SC], F32, tag="ysb", bufs=3)
                            nc.vector.tensor_copy(y_sb[:], y_ps[:])
                            nc.sync.dma_start(
                                yT_d[mi * 128:(mi + 1) * 128,
                                     sc_i * SC:(sc_i + 1) * SC],
                                y_sb[:])

    nc.compile()
    return nc


def _host_inputs(x, Wq, Wk, Wv, Wo):
    """Shard + pre-transpose/permute inputs on the host (free: not HW time)."""
    x = np.asarray(x, dtype=np.float32)
    WqT = np.ascontiguousarray(np.asarray(Wq, np.float32).T)   # [H, 2048]
    WkT = np.ascontiguousarray(np.asarray(Wk, np.float32).T)   # [H, 512]
    WvT = np.ascontiguousarray(np.asarray(Wv, np.float32).T)
    WoT = np.ascontiguousarray(np.asarray(Wo, np.float32).T)   # [H, 2048]

    # causal band mask: M[k, t] = 0 if t >= k + 384 else NEG
    kk = np.arange(128)[:, None]
    tt = np.arange(384 + SC)[None, :]
    mask = np.where(tt >= kk + 384, 0.0, NEG).astype(np.float32)
    ident = np.zeros((128, 64), np.float32)
    ident[np.arange(128), np.arange(128) % 64] = 1.0  # two stacked I_64
    vc0 = np.zeros((128, ST * 65), np.float32)
    vc0.reshape(128, ST, 65)[:, :, 64] = 1.0
    vc1 = np.zeros((128, ST * 128), np.float32)
    vc1.reshape(128, ST, 128)[:, :, 0] = 1.0
    ones_row = np.ones((1, 128), np.float32)

    # head permutation inside a group: natural row kh*256+gg*64+d
    # -> stored row gg*128+kh*64+d
    perm = np.empty(DQ, np.int64)
    for kh in range(2):
        for gg in range(4):
            src = kh * 256 + gg * 64
            dst = gg * 128 + kh * 64
            perm[dst:dst + 64] = np.arange(src, src + 64)

    in_maps = []
    for c in range(N_CORES):
        b, g = c // NGROUPS, c % NGROUPS
        xT = np.ascontiguousarray(x[b].T)                      # [H, S]
        wq_p = np.ascontiguousarray(WqT[:, g * DQ:(g + 1) * DQ][:, perm])
        wk_g = np.ascontiguousarray(WkT[:, g * DKV:(g + 1) * DKV])
        wv_g = np.ascontiguousarray(WvT[:, g * DKV:(g + 1) * DKV])
        wo_p = np.ascontiguousarray(WoT[g * DQ:(g + 1) * DQ, :][perm, :])
        in_maps.append({"xT": xT, "wq": wq_p, "wk": wk_g, "wv": wv_g,
                        "wo": wo_p, "mask": mask, "ident": ident,
                        "vc0": vc0, "vc1": vc1, "ones": ones_row})
    return in_maps


_NC_CACHE = {}


def _get_nc(loops=1):
    if loops not in _NC_CACHE:
        _NC_CACHE[loops] = build_gqa(loops)
    return _NC_CACHE[loops]


def kernel(x, Wq, Wk, Wv, Wo):
    nc = _get_nc(1)
    in_maps = _host_inputs(x, Wq, Wk, Wv, Wo)
    res = bass_utils.run_bass_kernel_spmd(
        nc, in_maps, core_ids=list(range(N_CORES)))
    # unshard: sum the row-parallel partials per batch, transpose back
    y = np.empty((B, S, H), np.float32)
    for b in range(B):
        acc = res.results[b * NGROUPS]["yT"].astype(np.float32).copy()
        for g in range(1, NGROUPS):
            acc += res.results[b * NGROUPS + g]["yT"]
        y[b] = acc.T
    return y


if __name__ == "__main__":
    rng = np.random.default_rng(0)
    x = rng.standard_normal((B, S, H), dtype=np.float32)
    Wq = (rng.standard_normal((H, H)) * 0.02).astype(np.float32)
    Wk = (rng.standard_normal((512, H)) * 0.02).astype(np.float32)
    Wv = (rng.standard_normal((512, H)) * 0.02).astype(np.float32)
    Wo = (rng.standard_normal((H, H)) * 0.02).astype(np.float32)
    y = kernel(x=x, Wq=Wq, Wk=Wk, Wv=Wv, Wo=Wo)
    print("ok", y.shape, float(np.abs(y).max()))


# revision 14
# speedup vs baseline: 1600.9502x; 1600.9502x over previous
"""Grouped-Query Attention kernel for 8 Trainium2 NeuronCores.

Problem (hardcoded): x [2,2048,2048] f32, Wq [2048,2048], Wk/Wv [512,2048],
Wo [2048,2048]; 32 q-heads, 8 kv-heads, head_dim 64, causal softmax.

Sharding: hybrid tensor-parallel = (batch 2) x (head-group 4). Core c handles
batch b=c//4, head-group g=c%4: 8 q-heads, 2 kv-heads, row-parallel Wo slice.
Each core emits a partial yT [H, S]; host sums the 4 partials per batch
(the all-reduce of the row-parallel projection) and transposes back.

All activations live transposed on-chip (feature dim on partitions) so every
matmul has its contraction on the partition axis:
  QT = wq_p.T @ xT   (per head-group, head-permuted columns)
  scoresT[k,q] = (KT.T QT) -- lhsT = KT slice, rhs = QT slice, K=64
  PT = exp(0.125*scoresT + causal mask)   (fully-masked key tiles skipped)
  O' = V_aug.T @ PT  -- V augmented with a ones column => softmax denominator
  yT_partial = woTs_p @ OT_norm

Structure/perf notes:
- Matmuls run in float32r (TF32-class, ~1.7e-4 rel err, 4x fp32 throughput;
  operands must be produced rounded-to-f32r, hence f32r tiles everywhere).
- The two kv-heads' K=64 score matmuls are issued back-to-back with lhsT at
  partition bases 0/64, so they land in disjoint PE row-groups and overlap.
- Host-side head permutation (q rows gg*128+kh*64) keeps lhsT/rhs partition
  bases equal everywhere (bass requires it) with zero on-chip data movement;
  the same permutation is applied to Wo rows so the projection contracts the
  permuted OT directly.
- The output projection for a seq chunk is fused right after that chunk's
  attention, so its PE work hides under the next chunk's softmax (ScalarE).
- V_aug kv-head-1 uses layout [ones | 0*63 | v] so its PV output rows land
  at partitions 64-127, letting the VectorE normalize write OT base-aligned.
"""

import numpy as np
import concourse.mybir as mybir
import concourse.tile as tile
from concourse import bacc, bass_utils

# ---- problem constants ----
B, S, H = 2, 2048, 2048
NKV, NQ, HD = 8, 32, 64
G = NQ // NKV              # 4 q-heads per kv-head
N_CORES = 8
NGROUPS = 4                # head-groups (cores per batch)
DQ = 512                   # q-proj rows per core (8 heads x 64)
DKV = 128                  # kv rows per core (2 kv heads x 64)
SC = 512                   # free-dim chunk (psum bank width in fp32)
NSC = S // SC              # 4
HT = H // 128              # 16 contraction tiles
ST = S // 128              # 16 key tiles
NEG = -1.0e5               # causal mask fill (exp(0.125*-1e5) == 0)

F32 = mybir.dt.float32
F32R = mybir.dt.float32r


def build_gqa(loops: int = 1, phases=(1, 2, 3)):
    """Build + bacc-compile the per-core kernel (SPMD: same program, cores
    differ only in input data)."""
    nc = bacc.Bacc("TRN2", target_bir_lowering=False, debug=False,
                   num_devices=N_CORES)

    # DRAM I/O (per-core shapes). float32r tensors carry fp32 bits; the PE
    # rounds them on read.
    xT_d = nc.dram_tensor("xT", [H, S], F32R, kind="ExternalInput")
    wq_d = nc.dram_tensor("wq", [H, DQ], F32R, kind="ExternalInput")
    wk_d = nc.dram_tensor("wk", [H, DKV], F32R, kind="ExternalInput")
    wv_d = nc.dram_tensor("wv", [H, DKV], F32R, kind="ExternalInput")
    wo_d = nc.dram_tensor("wo", [DQ, H], F32R, kind="ExternalInput")
    mask_d = nc.dram_tensor("mask", [128, 384 + SC], F32, kind="ExternalInput")
    ident_d = nc.dram_tensor("ident", [128, 64], F32R, kind="ExternalInput")
    vc0_d = nc.dram_tensor("vc0", [128, ST * 65], F32R, kind="ExternalInput")
    vc1_d = nc.dram_tensor("vc1", [128, ST * 128], F32R, kind="ExternalInput")
    ones_d = nc.dram_tensor("ones", [1, 128], F32R, kind="ExternalInput")
    yT_d = nc.dram_tensor("yT", [H, S], F32, kind="ExternalOutput")

    with tile.TileContext(nc) as tc:
        with tc.tile_pool(name="persist", bufs=1) as pp:
            # persistent SBUF (per-partition bytes in comments)
            qt = [pp.tile([128, S], F32R, tag=f"qt{i}", name=f"qt{i}") for i in range(4)]   # 32K
            kt = pp.tile([128, S], F32R, tag="kt")                            # 8K
            ot = [pp.tile([128, S], F32R, tag=f"ot{i}", name=f"ot{i}") for i in range(4)]   # 32K
            # V_aug: kh=0 layout [v(64) | ones] width 65 (O rows 0-63, den 64)
            #        kh=1 layout [ones | zeros(63) | v(64)] width 128
            #        (den row 0, O rows 64-127)
            vaug0 = pp.tile([128, ST * 65], F32R, tag="vaug0")               # 4.1K
            vaug1 = pp.tile([128, ST * 128], F32R, tag="vaug1")              # 8K
            mask_t = pp.tile([128, 384 + SC], F32, tag="mask")               # 3.5K
            ident_t = pp.tile([128, 64], F32R, tag="ident")                  # 0.25K
            ones_t = pp.tile([1, 128], F32R, tag="ones")                     # tiny

            nc.sync.dma_start(mask_t[:], mask_d[:])
            nc.sync.dma_start(ident_t[:], ident_d[:])
            nc.sync.dma_start(ones_t[:], ones_d[:])
            # constant columns of V_aug (ones/zeros pattern from host)
            nc.sync.dma_start(vaug0[:], vc0_d[:])
            nc.sync.dma_start(vaug1[:], vc1_d[:])

            for _rep in range(loops):
                # ============ Phase 1: QKV projections (+ V transpose) ======
                with (
                    tc.tile_pool(name="p1sb", bufs=1) as p1,
                    tc.tile_pool(name="p1ps", bufs=1, space="PSUM") as ps1,
                ):
                    wq_t = p1.tile([128, HT * DQ], F32R, tag="wq")           # 32K
                    wk_t = p1.tile([128, HT * DKV], F32R, tag="wk")          # 8K
                    wv_t = p1.tile([128, HT * DKV], F32R, tag="wv")          # 8K
                    nc.sync.dma_start(
                        wq_t[:].rearrange("p (t m) -> p t m", t=HT),
                        wq_d[:].rearrange("(t p) m -> p t m", p=128))
                    nc.sync.dma_start(
                        wk_t[:].rearrange("p (t m) -> p t m", t=HT),
                        wk_d[:].rearrange("(t p) m -> p t m", p=128))
                    nc.sync.dma_start(
                        wv_t[:].rearrange("p (t m) -> p t m", t=HT),
                        wv_d[:].rearrange("(t p) m -> p t m", p=128))

                    for sq in range(NSC):
                        xq = [p1.tile([128, SC], F32R, tag="xq", name=f"xq{ht}", bufs=20) for ht in range(HT)]
                        for ht in range(HT):
                            nc.sync.dma_start(
                                xq[ht][:],
                                xT_d[ht * 128:(ht + 1) * 128, sq * SC:(sq + 1) * SC])
                        # 6 output m-tiles: 4x QT, 1x KT, 1x VT
                        vt_tmp = p1.tile([128, SC], F32R, tag="vt", bufs=2)
                        jobs = [(wq_t, DQ, mi, qt[mi]) for mi in range(4)]
                        jobs.append((wk_t, DKV, 0, kt))
                        jobs.append((wv_t, DKV, 0, None))  # None -> vt_tmp
                        for w_t, mw, mi, dest in jobs:
                            psum = ps1.tile([128, SC], F32, tag="qkv", bufs=4)
                            for ht in range(HT):
                                nc.tensor.matmul(
                                    psum[:],
                                    w_t[:, ht * mw + mi * 128: ht * mw + mi * 128 + 128],
                                    xq[ht][:],
                                    start=(ht == 0), stop=(ht == HT - 1))
                            if dest is None:
                                nc.vector.tensor_copy(vt_tmp[:], psum[:])
                            else:
                                nc.vector.tensor_copy(
                                    dest[:, sq * SC:(sq + 1) * SC], psum[:])
                        # V transpose for this quarter's 4 key-tiles
                        for stq in range(4):
                            st = sq * 4 + stq
                            for kh in range(2):
                                tp = ps1.tile([128, 64], F32R, tag="tr", bufs=4)
                                nc.tensor.transpose(
                                    tp[:],
                                    vt_tmp[kh * 64:(kh + 1) * 64,
                                           stq * 128:(stq + 1) * 128],
                                    ident_t[kh * 64:(kh + 1) * 64, :])
                                if kh == 0:
                                    nc.vector.tensor_copy(
                                        vaug0[:, st * 65: st * 65 + 64], tp[:])
                                else:
                                    nc.vector.tensor_copy(
                                        vaug1[:, st * 128 + 64: st * 128 + 128], tp[:])

                if 2 not in phases:
                    continue
                # ============ Phase 2+3 pools ===============================
                with (
                    tc.tile_pool(name="p2sb", bufs=1) as p2,
                    tc.tile_pool(name="p2ps", bufs=1, space="PSUM") as ps2,
                ):
                    wo_t = p2.tile([128, 4 * H], F32R, tag="wo")             # 32K
                    nc.sync.dma_start(
                        wo_t[:].rearrange("p (t m) -> p t m", t=4),
                        wo_d[:].rearrange("(t p) m -> p t m", p=128))

                    # ---- Phase 2: causal attention, kv-head-paired so the
                    # two K=64 score matmuls occupy disjoint PE row groups
                    # (rows 0-63 / 64-127) and run concurrently; the chunk's
                    # output projection follows immediately (phase 3 fused).
                    for qc in range(NSC):
                        for gg in range(4):
                            n_st = 4 * (qc + 1)
                            o_pair = [ps2.tile([128, SC], F32, tag="o",
                                               name=f"o{kh}", bufs=3)
                                      for kh in range(2)]
                            for st in range(n_st):
                                dd = st - 4 * qc
                                pts = []
                                for kh in range(2):
                                    q_ap = qt[gg][kh * 64:(kh + 1) * 64,
                                                  qc * SC:(qc + 1) * SC]
                                    sc_ps = ps2.tile([128, SC], F32, tag="sc",
                                                     name=f"s{kh}", bufs=4)
                                    nc.tensor.matmul(
                                        sc_ps[:],
                                        kt[kh * 64:(kh + 1) * 64,
                                           st * 128:(st + 1) * 128],
                                        q_ap, start=True, stop=True)
                                    if dd >= 0:  # diagonal band tile
                                        nc.vector.tensor_add(
                                            sc_ps[:], sc_ps[:],
                                            mask_t[:, 384 - dd * 128:
                                                   896 - dd * 128])
                                    pt = p2.tile([128, SC], F32R, tag="pt",
                                                 name=f"pt{kh}", bufs=6)
                                    nc.scalar.activation(
                                        pt[:], sc_ps[:],
                                        mybir.ActivationFunctionType.Exp,
                                        scale=0.125)
                                    pts.append(pt)
                                nc.tensor.matmul(
                                    o_pair[0][0:65, :],
                                    vaug0[:, st * 65: st * 65 + 65],
                                    pts[0][:],
                                    start=(st == 0), stop=(st == n_st - 1))
                                nc.tensor.matmul(
                                    o_pair[1][:],
                                    vaug1[:, st * 128: st * 128 + 128],
                                    pts[1][:],
                                    start=(st == 0), stop=(st == n_st - 1))
                            # normalize rows by the ones-column sums
                            for kh in range(2):
                                o_ps = o_pair[kh]
                                den_row = o_ps[64:65, :] if kh == 0 else o_ps[0:1, :]
                                o_rows = o_ps[0:64, :] if kh == 0 else o_ps[64:128, :]
                                rec32 = p2.tile([1, SC], F32, tag="rec32", bufs=2)
                                nc.vector.reciprocal(rec32[:], den_row)
                                rec_r = p2.tile([1, SC], F32R, tag="recr", bufs=2)
                                nc.vector.tensor_copy(rec_r[:], rec32[:])
                                rb_ps = ps2.tile([128, SC], F32, tag="rb", bufs=1)
                                nc.tensor.matmul(rb_ps[:], ones_t[:], rec_r[:],
                                                 start=True, stop=True)
                                rb_sb = p2.tile([128, SC], F32, tag="rbsb", bufs=2)
                                nc.vector.tensor_copy(
                                    rb_sb[kh * 64:(kh + 1) * 64, :],
                                    rb_ps[kh * 64:(kh + 1) * 64, :])
                                nc.vector.tensor_mul(
                                    ot[gg][kh * 64:(kh + 1) * 64,
                                           qc * SC:(qc + 1) * SC],
                                    o_rows, rb_sb[kh * 64:(kh + 1) * 64, :])

                        # ---- fused phase 3: project this chunk's columns ----
                        for mi in range(HT if 3 in phases else 0):
                            y_ps = ps2.tile([128, SC], F32, tag="sc", name="y_ps", bufs=4)
                            for kt_i in range(4):
                                nc.tensor.matmul(
                                    y_ps[:],
                                    wo_t[:, kt_i * H + mi * 128:
                                         kt_i * H + mi * 128 + 128],
                                    ot[kt_i][:, qc * SC:(qc + 1) * SC],
                                    start=(kt_i == 0), stop=(kt_i == 3))
                            y_sb = p2.tile([128, SC], F32, tag="ysb", bufs=3)
                            nc.vector.tensor_copy(y_sb[:], y_ps[:])
                            nc.sync.dma_start(
                                yT_d[mi * 128:(mi + 1) * 128,
                                     qc * SC:(qc + 1) * SC],
                                y_sb[:])

    nc.compile()
    return nc


def _host_inputs(x, Wq, Wk, Wv, Wo):
    """Shard + pre-transpose/permute inputs on the host (free: not HW time)."""
    x = np.asarray(x, dtype=np.float32)
    WqT = np.ascontiguousarray(np.asarray(Wq, np.float32).T)   # [H, 2048]
    WkT = np.ascontiguousarray(np.asarray(Wk, np.float32).T)   # [H, 512]
    WvT = np.ascontiguousarray(np.asarray(Wv, np.float32).T)
    WoT = np.ascontiguousarray(np.asarray(Wo, np.float32).T)   # [H, 2048]

    # causal band mask: M[k, t] = 0 if t >= k + 384 else NEG
    kk = np.arange(128)[:, None]
    tt = np.arange(384 + SC)[None, :]
    mask = np.where(tt >= kk + 384, 0.0, NEG).astype(np.float32)
    ident = np.zeros((128, 64), np.float32)
    ident[np.arange(128), np.arange(128) % 64] = 1.0  # two stacked I_64
    vc0 = np.zeros((128, ST * 65), np.float32)
    vc0.reshape(128, ST, 65)[:, :, 64] = 1.0
    vc1 = np.zeros((128, ST * 128), np.float32)
    vc1.reshape(128, ST, 128)[:, :, 0] = 1.0
    ones_row = np.ones((1, 128), np.float32)

    # head permutation inside a group: natural row kh*256+gg*64+d
    # -> stored row gg*128+kh*64+d
    perm = np.empty(DQ, np.int64)
    for kh in range(2):
        for gg in range(4):
            src = kh * 256 + gg * 64
            dst = gg * 128 + kh * 64
            perm[dst:dst + 64] = np.arange(src, src + 64)

    in_maps = []
    for c in range(N_CORES):
        b, g = c // NGROUPS, c % NGROUPS
        xT = np.ascontiguousarray(x[b].T)                      # [H, S]
        wq_p = np.ascontiguousarray(WqT[:, g * DQ:(g + 1) * DQ][:, perm])
        wk_g = np.ascontiguousarray(WkT[:, g * DKV:(g + 1) * DKV])
        wv_g = np.ascontiguousarray(WvT[:, g * DKV:(g + 1) * DKV])
        wo_p = np.ascontiguousarray(WoT[g * DQ:(g + 1) * DQ, :][perm, :])
        in_maps.append({"xT": xT, "wq": wq_p, "wk": wk_g, "wv": wv_g,
                        "wo": wo_p, "mask": mask, "ident": ident,
                        "vc0": vc0, "vc1": vc1, "ones": ones_row})
    return in_maps


_NC_CACHE = {}


def _get_nc(loops=1):
    if loops not in _NC_CACHE:
        _NC_CACHE[loops] = build_gqa(loops)
    return _NC_CACHE[loops]


def kernel(x, Wq, Wk, Wv, Wo):
    nc = _get_nc(1)
    in_maps = _host_inputs(x, Wq, Wk, Wv, Wo)
    res = bass_utils.run_bass_kernel_spmd(
        nc, in_maps, core_ids=list(range(N_CORES)))
    # unshard: sum the row-parallel partials per batch, transpose back
    y = np.empty((B, S, H), np.float32)
    for b in range(B):
        acc = res.results[b * NGROUPS]["yT"].astype(np.float32).copy()
        for g in range(1, NGROUPS):
            acc += res.results[b * NGROUPS + g]["yT"]
        y[b] = acc.T
    return y


if __name__ == "__main__":
    rng = np.random.default_rng(0)
    x = rng.standard_normal((B, S, H), dtype=np.float32)
    Wq = (rng.standard_normal((H, H)) * 0.02).astype(np.float32)
    Wk = (rng.standard_normal((512, H)) * 0.02).astype(np.float32)
    Wv = (rng.standard_normal((512, H)) * 0.02).astype(np.float32)
    Wo = (rng.standard_normal((H, H)) * 0.02).astype(np.float32)
    y = kernel(x=x, Wq=Wq, Wk=Wk, Wv=Wv, Wo=Wo)
    print("ok", y.shape, float(np.abs(y).max()))
